# revision 29
# baseline (speedup 1.0000x reference)
"""CovPool kernel for 8 TRN2 NeuronCores.

reference semantics (B=32, N=16384, D=64):
    cov_b = (X_b - mean_b)^T (X_b - mean_b) / (N-1) + lam*I        (64x64)
    out   = sort(concat_b triu(cov_b)) reshaped to (B, 2080)

Device strategy (data parallel over batch, core c owns batches [4c, 4c+4)):
  per-core slab = 16 MiB, streamed once (DMA floor ~47 us @ 360 GB/s).
  - SP issues the big input DMAs (1 MiB each; nothing else on SP).
  - Activation casts each f32 buffer to bf16 (feeds the PE).
  - PE accumulates G = X^T X per batch: one [128,64]x[128,64] bf16 Gram
    matmul per 128-row chunk into PSUM.
  - DVE reduces per-buffer column partial sums (r-strided tensor_reduce),
    tree-adds them, then PE folds partitions via a ones-matmul and applies
    the rank-1 mean correction -s s^T / N into PSUM.
  - DVE scales PSUM to cov + lam*I into a shared [64, 4*64] tile; one
    merged output DMA (issued from DVE) writes all 4 covs.
  - host: triu extract + global sort (tiny: 32x2080 elements).
  The final batch's stream is tapered (smaller trailing DMAs) to cut the
  pipeline drain after the last transfer.
"""

import sys

sys.path.insert(0, "/opt/trn_rl_repo")

import numpy as np

from concourse import bacc, mybir
from concourse.tile import TileContext
from concourse.bass_utils import run_bass_kernel_spmd

B, N, D = 32, 16384, 64
NCORES = 8
BPC = B // NCORES  # batches per core
LAMBDA = 0.01
D_OUT = D * (D + 1) // 2  # 2080

MAX_R = 32                  # max rows per partition per stream buffer
MAX_FREE = MAX_R * D        # 2048 f32 per partition = 8 KB (1 MiB DMA)
NSTREAM = 6                 # stream ring depth
SUMW = 512                  # ones-matmul quarter width (one PSUM bank row)

# per-batch segment plans, in rows-per-partition units (sum = 128 per batch
# since 128 partitions * 128 r-units = 16384 rows)
FULL_PLAN = [32, 32, 32, 32]
TAPER_PLAN = [32, 32, 24, 16, 8, 8, 4, 2, 2]  # last batch: drain taper

f32 = mybir.dt.float32
bf16 = mybir.dt.bfloat16


def _emit_body(tc, nc, x, out, stream, work_pool, psum_pool):
    """One full covariance pass over this core's BPC batches.

    Each batch's tail (mean fold + rank-1 correction + cov scale) is
    deferred into the NEXT batch's stream so its inputs are long ready by
    the time the in-order PE/DVE queues reach it — no sequencer stalls.
    """
    di = 0  # global stream-slot counter
    stream_f32, stream_bf = stream
    gram, ones_col = out
    xf = x.rearrange("b n d -> b (n d)")  # flat per-batch view

    def tail(b, psum):
        # raw Gram + column sums: PSUM -> SBUF bounce -> HBM.
        # psum row D holds quartered column sums (8 r-groups x 64); host
        # finishes the fold + rank-1 mean correction. Two parallel chains:
        # Pool copies the Gram rows, DVE folds the sums.
        gram_sb = work_pool.tile([D + 1, D], f32, tag=f"gram{b % 2}")
        nc.scalar.copy(gram_sb[0:D, 0:D], psum[0:D, 0:D])
        nc.vector.tensor_reduce(
            out=gram_sb[D:D + 1, 0:D],
            in_=psum[D:D + 1, 0:SUMW].rearrange("p (q d) -> p d q", d=D),
            axis=mybir.AxisListType.X, op=mybir.AluOpType.add,
        )
        if b == BPC - 1:
            # final batch: SP's input queue is drained, shortest DGE path
            nc.sync.dma_start(gram[b], gram_sb[:])
        else:
            # mid-stream: issue from Pool (SWDGE) so neither SP's input
            # stream nor Act's cast queue is ever held up by this wait
            nc.gpsimd.dma_start(gram[b], gram_sb[:])

    prev = None  # (b, psum) awaiting tail emission
    for b in range(BPC):
        plan = TAPER_PLAN if b == BPC - 1 else FULL_PLAN
        psum = psum_pool.tile([D + 1, 512], f32, tag=f"acc{b % BPC}")
        last = (len(plan) - 1, plan[-1] - 1)
        row0 = 0  # running row offset
        qi = 0  # ones-matmul quarter counter (for start flags)
        nq_total = sum(-(-r * D // SUMW) for r in plan)
        for t, r_per_part in enumerate(plan):
            free = r_per_part * D
            nelem = 128 * r_per_part * D
            buf = stream_f32[di % NSTREAM]
            bbuf = stream_bf[di % NSTREAM]
            di += 1
            nc.sync.dma_start(
                buf[:, 0:free],
                xf[b, row0 * D:row0 * D + nelem]
                .rearrange("(p f) -> p f", p=128),
            )
            row0 += 128 * r_per_part
            # fp32 -> bf16 cast, alternating Act / DVE so neither engine's
            # backlog sits on the drain path; parity chosen so the final
            # segment's cast runs on DVE (faster per element)
            if b == BPC - 1:
                on_act = (len(plan) - 1 - t) % 2 == 1
            else:
                on_act = di % 2 == 1
            if on_act:
                nc.scalar.copy(bbuf[:, 0:free], buf[:, 0:free])
            else:
                nc.vector.tensor_copy(bbuf[:, 0:free], buf[:, 0:free])
            for r in range(r_per_part):
                nc.tensor.matmul(
                    psum[0:D, 0:D], bbuf[:, r * D:(r + 1) * D],
                    bbuf[:, r * D:(r + 1) * D],
                    start=(t == 0 and r == 0), stop=((t, r) == last),
                )
            # column sums on PE: ones^T @ quarter -> psum row D accumulates
            # 8 r-group partial sums per d (folded later)
            for q0 in range(0, free, SUMW):
                w = min(SUMW, free - q0)
                nc.tensor.matmul(
                    psum[D:D + 1, 0:w], ones_col[:], bbuf[:, q0:q0 + w],
                    start=(qi == 0), stop=(qi == nq_total - 1),
                )
                qi += 1
            if t == 0 and prev is not None:
                tail(*prev)
                prev = None
        prev = (b, psum)
    tail(*prev)


def build_cov_kernel(bench_reps=None, variant="full"):
    assert variant == "full"
    nc = bacc.Bacc("TRN2", target_bir_lowering=False, debug=False,
                   num_devices=NCORES)
    x = nc.dram_tensor("x", [BPC, N, D], f32, kind="ExternalInput")
    gram = nc.dram_tensor("gram", [BPC, D + 1, D], f32,
                          kind="ExternalOutput")

    with TileContext(nc) as tc:
        with (
            tc.tile_pool(name="stream", bufs=1) as stream_pool,
            tc.tile_pool(name="const", bufs=1) as const_pool,
            tc.tile_pool(name="work", bufs=2) as work_pool,
            tc.tile_pool(name="psum", bufs=1, space="PSUM") as psum_pool,
        ):
            ones_col = const_pool.tile([128, 1], bf16, tag="ones")
            nc.vector.memset(ones_col[:], 1.0)
            stream_f32 = [
                stream_pool.tile([128, MAX_FREE], f32,
                                 tag=f"stream{i}", name=f"stream{i}")
                for i in range(NSTREAM)
            ]
            stream_bf = [
                stream_pool.tile([128, MAX_FREE], bf16,
                                 tag=f"streambf{i}", name=f"streambf{i}")
                for i in range(NSTREAM)
            ]
            stream = (stream_f32, stream_bf)

            def body():
                _emit_body(tc, nc, x, (gram, ones_col), stream,
                           work_pool, psum_pool)

            if bench_reps is None:
                body()
            else:
                with tc.For_i(0, bench_reps, 1):
                    body()

    nc.compile()
    return nc


_NC_CACHE = {}


def _get_kernel():
    if "nc" not in _NC_CACHE:
        _NC_CACHE["nc"] = build_cov_kernel()
    return _NC_CACHE["nc"]


def make_in_maps(x_full: np.ndarray):
    return [
        {"x": np.ascontiguousarray(x_full[c * BPC:(c + 1) * BPC])}
        for c in range(NCORES)
    ]


def postprocess(results):
    """results: list of per-core out dicts -> final (B, D_OUT) array.

    Device ships raw Gram G = X^T X and per-partition column sums; the
    tiny mean correction / scale / +lam*I runs here.
    """
    raw = np.concatenate([results[c]["gram"] for c in range(NCORES)],
                         axis=0)  # (B, D+1, D)
    G = raw[:, 0:D, :]
    s = raw[:, D, :]  # (B, D)
    cov = (G - s[:, :, None] * s[:, None, :] / N) / (N - 1)
    cov += LAMBDA * np.eye(D, dtype=np.float32)[None]
    iu, ju = np.triu_indices(D)
    tri = cov[:, iu, ju]  # (B, D_OUT)
    return np.sort(tri.reshape(-1)).reshape(B, D_OUT).astype(np.float32)


def run_device(x_full: np.ndarray):
    nc = _get_kernel()
    res = run_bass_kernel_spmd(nc, make_in_maps(x_full),
                               core_ids=list(range(NCORES)))
    return res.results


def kernel(x: np.ndarray) -> np.ndarray:
    x = np.asarray(x, dtype=np.float32)
    return postprocess(run_device(x))


if __name__ == "__main__":
    rng = np.random.default_rng(0)
    xt = rng.standard_normal((B, N, D), dtype=np.float32)
    o = kernel(xt)
    print("kernel out shape:", o.shape, o.dtype)
